# revision 1
# baseline (speedup 1.0000x reference)
"""Multi-head attention (B=8, N=1024, C=768, H=12) on 8 TRN2 NeuronCores.

Sharding: pure data parallel — batch element b runs on core b. Each core
computes the full attention block for its [1024, 768] slice; no collectives.

Per-core dataflow (everything "transposed" so the contraction dim always
lands on SBUF partitions):
  xT [C, N] (host-pre-transposed, bf16)
  qT/kT chunks  = w_qkvT_chunk.T @ xT        -> [128, N] per head-pair
  v             = xT_chunk.T @ w_vT          -> [N, 768] (m on partitions)
  sT (per head) = kT.T @ qT                  -> [N, N], two heads packed in
                  one PE pass via row-group tile_position (K=64 each)
  exp           = ScalarE Exp(scale=1/8) psum->sbuf bf16
  o_unT/denom   = [v_h | 1].T @ exp_sT       -> [65, N]  (M=65: row 64 is
                  the softmax denominator, so no separate reduction pass)
  r = 1/denom; broadcast across partitions via a K=1 matmul with ones
  oT = o_unT * r; y = proj(oT) + bias        -> [N, C] fp32 out

Emission order forms a software pipeline: pair j's AV and pair j+1's qT/kT
production fill PE gaps while ScalarE (the bottleneck) works through pair
j's exp tiles.

The single-wait legalizer below works around this container's walrus build,
which refuses instructions carrying more than one semaphore wait (the TPB
instruction encoding has exactly one wait slot; this walrus does not split).
"""

import sys

for _p in ("/opt/trn_rl_repo", "/root/.axon_site/_ro/trn_rl_repo"):
    if _p not in sys.path:
        sys.path.append(_p)

import numpy as np
import ml_dtypes

import concourse.bass as bass
import concourse.tile as tile
from concourse import mybir
from concourse.bass_utils import run_bass_kernel_spmd

B, N, C = 8, 1024, 768
H, D = 12, 64
KT = C // 128       # 6 contraction tiles
NT = N // 128       # 8 sequence tiles
PAIRS = H // 2      # 6 head pairs
BF16 = mybir.dt.bfloat16
F32 = mybir.dt.float32
N_CORES = 8


def legalize_single_wait(nc):
    """Split multi-wait instructions into single-wait NoOps + instruction."""
    stats = {"split_insts": 0, "nops_added": 0, "multi_update": 0}
    for f in nc.m.functions:
        for blk in f.blocks:
            insts = blk.instructions
            if not any(
                i.sync_info is not None and len(i.sync_info.on_wait) > 1
                for i in insts
            ):
                continue
            new = []
            for inst in insts:
                si = inst.sync_info
                if si is not None and len(si.on_update) > 1:
                    stats["multi_update"] += 1
                if si is not None and len(si.on_wait) > 1:
                    waits = list(si.on_wait)
                    for k, w in enumerate(waits[:-1]):
                        nop = mybir.InstNoOp(
                            name=f"{inst.name}-swl{k}", ins=[], outs=[]
                        )
                        nop.engine = inst.engine
                        nop.sync_info = mybir.SyncInfo(on_wait=[w], on_update=[])
                        new.append(nop)
                        stats["nops_added"] += 1
                    inst.sync_info = mybir.SyncInfo(
                        on_wait=[waits[-1]], on_update=list(si.on_update)
                    )
                    stats["split_insts"] += 1
                new.append(inst)
            blk.instructions = new
    return stats


def build_attention_nc(repeat=1):
    nc = bass.Bass()
    xt_d = nc.dram_tensor("xt", [C, N], BF16, kind="ExternalInput")
    wq_d = nc.dram_tensor("wqkvt", [C, 3 * C], BF16, kind="ExternalInput")
    wp_d = nc.dram_tensor("wpt", [C, C], BF16, kind="ExternalInput")
    bias_d = nc.dram_tensor("biasb", [128, C], F32, kind="ExternalInput")
    y_d = nc.dram_tensor("y", [N, C], F32, kind="ExternalOutput")

    EXP = mybir.ActivationFunctionType.Exp

    with tile.TileContext(nc) as tc:
        with (
            tc.tile_pool(name="const", bufs=1) as cpool,
            tc.tile_pool(name="exp_sb", bufs=24) as epool,
            tc.tile_pool(name="small", bufs=4) as spool,
            tc.tile_pool(name="ysb", bufs=3) as ypool,
            tc.tile_pool(name="ps_qk", bufs=2, space="PSUM") as ps_qk,
            tc.tile_pool(name="ps_t", bufs=2, space="PSUM") as ps_t,
        ):
            # per-k-tile input DMAs so the first matmuls start early
            xt = cpool.tile([128, KT, N], BF16, name="xt_sb")
            wq = cpool.tile([128, KT, 3 * C], BF16, name="wq_sb")
            xt_r = xt_d.rearrange("(k p) n -> p k n", p=128)
            wq_r = wq_d.rearrange("(k p) o -> p k o", p=128)
            for k in range(KT):
                nc.sync.dma_start(out=wq[:, k, :], in_=wq_r[:, k, :])
                nc.sync.dma_start(out=xt[:, k, :], in_=xt_r[:, k, :])
            wp = cpool.tile([128, KT, C], BF16, name="wp_sb")
            nc.sync.dma_start(
                out=wp[:, :, :], in_=wp_d.rearrange("(k p) o -> p k o", p=128)
            )
            bias = cpool.tile([128, C], F32, name="bias_sb")
            nc.sync.dma_start(out=bias[:, :], in_=bias_d[:, :])
            ones_r = cpool.tile([1, 64], F32, name="ones_r")
            nc.vector.memset(ones_r[0:1, :], 1.0)
            v_all = cpool.tile([128, NT, H, 65], BF16, name="v_all")
            nc.vector.memset(v_all[:, :, :, 64:65], 1.0)
            oT = cpool.tile([128, PAIRS, N], BF16, name="oT_sb")
            qkT = cpool.tile([128, 2 * PAIRS, N], BF16, name="qkT_sb")

            def emit_qkprod(j):
                for half, woff in ((0, j * 128), (1, C + j * 128)):
                    qk_ps = ps_t.tile([128, 1024], F32, name="qk_ps", tag="pst")
                    for k in range(KT):
                        for n0 in (0, 512):
                            nc.tensor.matmul(
                                qk_ps[:, n0 : n0 + 512],
                                wq[:, k, woff : woff + 128],
                                xt[:, k, n0 : n0 + 512],
                                start=(k == 0),
                                stop=(k == KT - 1),
                            )
                    nc.vector.tensor_copy(
                        out=qkT[:, 2 * j + half, :], in_=qk_ps[:, :]
                    )

            def emit_v(m):
                # v = x @ w_v^T in [m(part), h, d] layout, plus a ones column
                v_ps = ps_t.tile([128, 1024], F32, name="v_ps", tag="pst")
                for k in range(KT):
                    for n0, nn_ in ((0, 512), (512, 256)):
                        nc.tensor.matmul(
                            v_ps[:, n0 : n0 + nn_],
                            xt[:, k, m * 128 : (m + 1) * 128],
                            wq[:, k, 2 * C + n0 : 2 * C + n0 + nn_],
                            start=(k == 0),
                            stop=(k == KT - 1),
                        )
                nc.vector.tensor_copy(
                    out=v_all[:, m, :, 0:64],
                    in_=v_ps[:, 0:C].rearrange("p (h d) -> p h d", h=H),
                )

            for _rep in range(repeat):
                emit_qkprod(0)

                for j in range(PAIRS):
                    qT = qkT[:, 2 * j, :]
                    kT_t = qkT[:, 2 * j + 1, :]
                    exp_tiles = []
                    for m in range(NT):
                        s_ps_a = ps_qk.tile([128, 1024], F32, name="s_ps_a", tag="qkps")
                        s_ps_b = ps_qk.tile([128, 1024], F32, name="s_ps_b", tag="qkps")
                        for n0 in (0, 512):
                            # two heads packed in PE row-groups (0,0) / (64,0)
                            nc.tensor.matmul(
                                s_ps_a[:, n0 : n0 + 512],
                                kT_t[0:64, m * 128 : (m + 1) * 128],
                                qT[0:64, n0 : n0 + 512],
                                start=True,
                                stop=True,
                            )
                            nc.tensor.matmul(
                                s_ps_b[:, n0 : n0 + 512],
                                kT_t[64:128, m * 128 : (m + 1) * 128],
                                qT[64:128, n0 : n0 + 512],
                                start=True,
                                stop=True,
                            )
                        ea = epool.tile([128, 1024], BF16, name="ea", tag="exp")
                        eb = epool.tile([128, 1024], BF16, name="eb", tag="exp")
                        nc.scalar.activation(
                            out=ea[:, :], in_=s_ps_a[:, :], func=EXP, scale=0.125
                        )
                        nc.scalar.activation(
                            out=eb[:, :], in_=s_ps_b[:, :], func=EXP, scale=0.125
                        )
                        exp_tiles.append((ea, eb))
                        if j == 0:
                            emit_v(m)

                    for hh in (0, 1):
                        h = 2 * j + hh
                        av_ps = ps_t.tile([128, 1024], F32, name="av_ps", tag="pst")
                        for m in range(NT):
                            e = exp_tiles[m][hh]
                            for n0 in (0, 512):
                                nc.tensor.matmul(
                                    av_ps[0:65, n0 : n0 + 512],
                                    v_all[:, m, h, :],
                                    e[:, n0 : n0 + 512],
                                    start=(m == 0),
                                    stop=(m == NT - 1),
                                )
                        r = spool.tile([1, 1024], F32, name="r", tag="r")
                        nc.vector.reciprocal(out=r[0:1, :], in_=av_ps[64:65, :])
                        bc_ps = ps_qk.tile([128, 1024], F32, name="bc_ps", tag="qkps")
                        for n0 in (0, 512):
                            nc.tensor.matmul(
                                bc_ps[0:64, n0 : n0 + 512],
                                ones_r[0:1, :],
                                r[0:1, n0 : n0 + 512],
                                start=True,
                                stop=True,
                            )
                        bc_sb = spool.tile([64, 1024], F32, name="bc_sb", tag="bc")
                        nc.vector.tensor_copy(out=bc_sb[0:64, :], in_=bc_ps[0:64, :])
                        nc.vector.tensor_mul(
                            out=oT[hh * 64 : (hh + 1) * 64, j, :],
                            in0=av_ps[0:64, :],
                            in1=bc_sb[0:64, :],
                        )
                    if j + 1 < PAIRS:
                        emit_qkprod(j + 1)

                # ---- projection + bias ----
                for nt in range(NT):
                    y_ps = ps_t.tile([128, 1024], F32, name="y_ps", tag="pst")
                    for p in range(PAIRS):
                        for n0, nn_ in ((0, 512), (512, 256)):
                            nc.tensor.matmul(
                                y_ps[:, n0 : n0 + nn_],
                                oT[:, p, nt * 128 : (nt + 1) * 128],
                                wp[:, p, n0 : n0 + nn_],
                                start=(p == 0),
                                stop=(p == PAIRS - 1),
                            )
                    y_sb = ypool.tile([128, C], F32, name="y_sb", tag="y")
                    nc.vector.tensor_add(out=y_sb[:, :], in0=y_ps[:, 0:C], in1=bias[:, :])
                    nc.sync.dma_start(
                        out=y_d[nt * 128 : (nt + 1) * 128, :], in_=y_sb[:, :]
                    )
    return nc


_NC_CACHE = None


def _get_nc(legalized=True):
    global _NC_CACHE
    if _NC_CACHE is None:
        nc = build_attention_nc()
        if legalized:
            legalize_single_wait(nc)
        _NC_CACHE = nc
    return _NC_CACHE


def _host_inputs(x, w_qkv, w_proj, b_proj):
    f32 = np.float32
    bf16 = ml_dtypes.bfloat16
    wqkvt = np.ascontiguousarray(np.asarray(w_qkv, f32).T).astype(bf16)
    wpt = np.ascontiguousarray(np.asarray(w_proj, f32).T).astype(bf16)
    biasb = np.ascontiguousarray(
        np.broadcast_to(np.asarray(b_proj, f32), (128, C))
    )
    x = np.asarray(x, f32)
    in_maps = []
    for b in range(N_CORES):
        xt = np.ascontiguousarray(x[b].T).astype(bf16)
        in_maps.append({"xt": xt, "wqkvt": wqkvt, "wpt": wpt, "biasb": biasb})
    return in_maps


def kernel(x, w_qkv, w_proj, b_proj):
    nc = _get_nc()
    in_maps = _host_inputs(x, w_qkv, w_proj, b_proj)
    res = run_bass_kernel_spmd(nc, in_maps, core_ids=list(range(N_CORES)))
    out = np.stack([r["y"] for r in res.results], axis=0)
    return np.ascontiguousarray(out.astype(np.float32))

